# revision 37
# baseline (speedup 1.0000x reference)
"""Trainium2 Bass kernel for nn_Block_CD (dual-stream patch-embed + attention).

Math per stream (x / y), tokens = (sample, l), l = 25 positions:
  xp  = conv3x3(img) + pos + conv_b           (im2col-ext matmul, K=52)
  xln = (xp - mu) * rstd                      (LN; g/b folded into qkv weights)
  qkv = (16 * qkv_w * g).T @ xln              (feature-major [768, tok])
  scores^T[(h,m),l] = Kbd_s^T @ q_s           (block-diag 4-head groups)
  E = exp(SCALE * scores); den = O^T @ E      (replicated over (h,d))
  av = Vbd_s^T @ E; avn = av / den
  out = proj^T @ avn + xp + (bp - pos - conv_b)
Output device layout: [512, B_loc*25] bf16, rearranged on host.
Sharding: pure data parallel, B=8192 over 8 cores.

Final structure (v12, sim 1.62 ms/core, HW-validated):
- hardware For_i loop (staggered_reset) over 1600-token chunks, 4 slices
  of 16 samples software-pipelined inside each body (phase stagger=2);
  33k-instruction unrolled baseline -> ~2.7k BIR instructions, which
  removes the dominant per-call host lowering/dispatch cost.
- single PSUM evacuation per embed half (ACT Copy feeds stats, residual
  and LN jointly); LN eps folded into the variance matmul as a
  1-partition accumulation row, keeping DVE (the critical engine) lean.
- LN mean fold: qkv = A @ im2col with A = (Wemb - mean) @ Wqkv^T, so the
  qkv matmuls contract 52 instead of 256 and need no centered activations.
- rstd via fast-inverse-sqrt bit trick + 1 Newton step (DVE/Pool), so the
  Activation engine runs a single function table ({Copy,Square,Exp}) and
  never pays the ~1.3us table-reload inside the loop.
- engine placement respects HW limits (GPSIMD cannot read PSUM; Pool has
  no TensorScalarPtr): PSUM consumers on ACT/DVE, SBUF-only block-diag
  builds and the residual+bias add on Pool.
"""
import sys
sys.path.insert(0, "/opt/trn_rl_repo")
import numpy as np
import ml_dtypes

import concourse.bass as bass
import concourse.mybir as mybir
import concourse.tile as tile
from concourse import bacc, bass_utils
from concourse.bass import ds

bf16 = mybir.dt.bfloat16
f32 = mybir.dt.float32
AF = mybir.ActivationFunctionType
ALU = mybir.AluOpType

DIM = 256
HEADS = 8
HD = 32
L = 25
SCALE = HD ** -0.5
LN_EPS = 1e-5
NCORES = 8
B = 8192
B_LOC = B // NCORES

S_I = 16          # samples per inner psum slice
N_I = S_I * L     # 400
U = 4             # slices unrolled per hw-loop iteration
STAGGER = 2       # phase offset between consecutive slices (sw pipeline)
TOK_B = U * N_I   # 800 tokens per iteration

_CACHE = {}


def _to_bf16(a):
    return np.asarray(a, np.float32).astype(ml_dtypes.bfloat16)


def _host_prep(inputs):
    pos = np.asarray(inputs["pos_embed"], np.float32).reshape(L, DIM)
    ln_g = np.asarray(inputs["ln_g"], np.float32)
    ln_b = np.asarray(inputs["ln_b"], np.float32)

    def im2col_ext(img):
        p = np.pad(np.asarray(img, np.float32), ((0, 0), (0, 0), (1, 1), (1, 1)))
        Bn = img.shape[0]
        cols = np.empty((Bn, L, 52), np.float32)
        idx = 0
        for c in range(3):
            for di in range(3):
                for dj in range(3):
                    cols[:, :, idx] = p[:, c, di:di + 5, dj:dj + 5].reshape(Bn, L)
                    idx += 1
        cols[:, :, 27:] = np.eye(L, dtype=np.float32)[None]
        return cols  # [B, 25, 52]

    prep = {}
    for nm, ik, cw, cb, qw, pw, pb in (
        ("x", "x", "conv1_w", "conv1_b", "qkv_x_w", "proj_x_w", "proj_x_b"),
        ("y", "y", "conv2_w", "conv2_b", "qkv_y_w", "proj_y_w", "proj_y_b"),
    ):
        conv_w = np.asarray(inputs[cw], np.float32)
        conv_b = np.asarray(inputs[cb], np.float32)
        qkv_w = np.asarray(inputs[qw], np.float32)
        proj_w = np.asarray(inputs[pw], np.float32)
        proj_b = np.asarray(inputs[pb], np.float32)

        w_emb = np.empty((52, DIM), np.float32)
        w_emb[:27] = conv_w.reshape(DIM, 27).T
        w_emb[27:] = pos + conv_b[None, :]
        prep[f"wemb_{nm}"] = w_emb

        wq = (16.0 * qkv_w * ln_g[None, :]).T          # [256, 768]
        w_emb_c = w_emb - w_emb.mean(axis=1, keepdims=True)
        prep[f"wqkv_{nm}"] = w_emb_c @ wq              # [52, 768] (LN mean folded)
        c = qkv_w @ ln_b
        assert np.abs(c).max() < 1e-6, "nonzero ln_b fold not supported"

        wp = proj_w.T                                   # [256, 256] lhsT
        prep[f"wproj_{nm}"] = np.concatenate([wp[0:128], wp[128:256]], axis=1)  # [128,512]

        fb = proj_b[:, None] - pos.T - conv_b[:, None]  # [256, 25]
        fbt = np.tile(fb, (1, S_I))                     # [256, 400]
        prep[f"fb_{nm}"] = np.concatenate([fbt[0:128], fbt[128:256]], axis=1)  # [128,800]

        prep[f"ic_{nm}"] = im2col_ext(inputs[ik])

    prep["ones_ln"] = np.full((128, 128), 1.0 / DIM, np.float32)
    O = np.zeros((128, 128), np.float32)
    for h in range(4):
        O[h * HD:h * HD + L, h * HD:(h + 1) * HD] = 1.0
    prep["ones_den"] = O
    return prep


def _slice_phases(nc, sb, ps, W, nm, ic, u, kbd, vbd, o2):
    """Return a list of phase-emitter closures for one 16-sample slice."""
    st = {}

    def ph_embed():
        st["xpf"] = [sb.tile([128, N_I], bf16, tag=f"xpf{t}{u}", name=f"xpf{t}{u}") for t in range(2)]
        st["sq"] = [sb.tile([128, N_I], bf16, tag=f"sq{t}{u}", name=f"sq{t}{u}") for t in range(2)]
        cl = slice(u * N_I, (u + 1) * N_I)
        for t in range(2):
            pt = ps.tile([128, N_I], f32, tag="mm", bufs=4)
            nc.tensor.matmul(pt[:, :], W[f"emb_{nm}"][:, 128 * t:128 * (t + 1)],
                             ic[:, cl], start=True, stop=True)
            nc.scalar.activation(st["xpf"][t][:, :], pt[:, :], AF.Copy)
            nc.scalar.activation(st["sq"][t][:, :], pt[:, :], AF.Square)

    def ph_stats():
        xp, sq = st["xpf"], st["sq"]
        rs = sb.tile([128, N_I], f32, tag=f"rs{u}", name=f"rs{u}")
        pm = ps.tile([128, N_I], f32, tag="mm", bufs=4)
        nc.tensor.matmul(pm[:, :], W["ones_ln"][:, :], xp[0][:, :], start=True, stop=False)
        nc.tensor.matmul(pm[:, :], W["ones_ln"][:, :], xp[1][:, :], start=False, stop=True)
        pv = ps.tile([128, N_I], f32, tag="mm", bufs=4)
        nc.tensor.matmul(pv[:, :], W["ones_ln"][:, :], sq[0][:, :], start=True, stop=False)
        nc.tensor.matmul(pv[:, :], W["ones_ln"][:, :], sq[1][:, :], start=False, stop=False)
        nc.tensor.matmul(pv[:, :], W["epsrow"][:, :], W["ones1"][:, :], start=False, stop=True)
        t1 = sb.tile([128, N_I], f32, tag=f"t1{u}", name=f"t1{u}")
        nc.scalar.activation(t1[:, :], pm[:, :], AF.Square)
        nc.vector.tensor_sub(t1[:, :], pv[:, :], t1[:, :])
        # rs = rsqrt(var + eps)/16 via fast-inverse-sqrt + 1 Newton step
        t2 = sb.tile([128, N_I], f32, tag=f"t2{u}", name=f"t2{u}")
        y0 = sb.tile([128, N_I], f32, tag=f"y0{u}", name=f"y0{u}")
        LSR = ALU.logical_shift_right
        nc.vector.tensor_scalar(y0[:, :].bitcast(mybir.dt.uint32),
                                t1[:, :].bitcast(mybir.dt.uint32), 1, None, LSR)
        nc.gpsimd.tensor_sub(y0[:, :].bitcast(mybir.dt.uint32),
                             W["magic"][:, :],
                             y0[:, :].bitcast(mybir.dt.uint32))
        nc.gpsimd.tensor_mul(t2[:, :], t1[:, :], y0[:, :])
        nc.gpsimd.tensor_mul(t2[:, :], t2[:, :], y0[:, :])
        nc.vector.tensor_scalar(t2[:, :], t2[:, :], -0.03125, 0.09375, ALU.mult, ALU.add)
        nc.gpsimd.tensor_mul(rs[:, :], y0[:, :], t2[:, :])
        st["rs"] = rs

    def ph_qkv():
        rs = st["rs"]
        cl = slice(u * N_I, (u + 1) * N_I)
        qkv = [sb.tile([128, N_I], bf16, tag=f"qkv{m}{u}", name=f"qkv{m}{u}") for m in range(4)]
        qv = sb.tile([128, 2 * 32 * S_I], bf16, tag=f"qv{u}", name=f"qv{u}")
        for g in range(2):
            nc.gpsimd.memset(
                qv[:, 512 * g:512 * (g + 1)].rearrange("p (s l) -> p s l", l=32)[:, :, L:32], 0.0)
        for m in range(6):
            pq = ps.tile([128, N_I], f32, tag="mm", bufs=4)
            nc.tensor.matmul(pq[:, :], W[f"qkv_{nm}"][:, 128 * m:128 * (m + 1)],
                             ic[:, cl], start=True, stop=True)
            if m in (0, 1):
                nc.vector.tensor_mul(qkv[m][:, :], pq[:, :], rs[:, :])
            elif m in (2, 3):
                nc.vector.tensor_mul(qkv[m][:, :], pq[:, :], rs[:, :])
            else:
                g = m - 4
                nc.vector.tensor_mul(
                    qv[:, 512 * g:512 * (g + 1)].rearrange("p (s l) -> p s l", l=32)[:, :, 0:L],
                    pq[:, :].rearrange("p (s l) -> p s l", l=L),
                    rs[:, :].rearrange("p (s l) -> p s l", l=L))
        st["qkv"] = qkv
        st["qv"] = qv

    def ph_trans():
        vt = sb.tile([128, 2 * 32 * S_I], bf16, tag=f"vt{u}", name=f"vt{u}")
        nc.vector.transpose(vt[:, :], st["qv"][:, :])
        st["vt"] = vt

    def ph_bd():
        vt, qkv = st["vt"], st["qkv"]
        for g in range(2):
            for h in range(4):
                kdst = (kbd[g][32 * h:32 * h + 32, :]
                        .rearrange("p (s m) -> p s m", m=128)[:, :, 32 * h:32 * h + L])
                ksrc = (qkv[2 + g][32 * h:32 * h + 32, :]
                        .rearrange("p (s m) -> p s m", m=L))
                if h != 3:
                    nc.gpsimd.tensor_copy(kdst, ksrc)
                else:
                    nc.scalar.activation(kdst, ksrc, AF.Copy)
        for h in range(4):
            vdst = (vbd[32 * h:32 * h + L, :]
                    .rearrange("p (g s d) -> p g s d", g=2, d=128)[:, :, :, 32 * h:32 * h + 32])
            vsrc = (vt[32 * h:32 * h + L, :]
                    .rearrange("p (g s d) -> p g s d", g=2, d=32))
            if h % 2 == 0:
                nc.scalar.activation(vdst, vsrc, AF.Copy)
            else:
                nc.gpsimd.tensor_copy(vdst, vsrc)

    def ph_attn():
        qkv = st["qkv"]
        avn = [sb.tile([128, N_I], bf16, tag=f"avn{g}{u}", name=f"avn{g}{u}") for g in range(2)]
        for g in range(2):
            sc = ps.tile([128, N_I], f32, tag="sc", bufs=3)
            for j in range(S_I):
                nc.tensor.matmul(
                    sc[0:128, j * L:(j + 1) * L],
                    kbd[g][:, 128 * j:128 * (j + 1)],
                    qkv[g][:, L * j:L * (j + 1)],
                    start=True, stop=True)
            ebuf = sb.tile([128, N_I], bf16, tag=f"e{g}{u}", name=f"e{g}{u}")
            nc.scalar.activation(ebuf[:, :], sc[:, :], AF.Exp, scale=SCALE)
            dn = ps.tile([128, N_I], f32, tag="mm", bufs=4)
            nc.tensor.matmul(dn[:, :], W["ones_den"][:, :], ebuf[:, :],
                             start=True, stop=True)
            rden = sb.tile([128, N_I], f32, tag=f"rden{u}", bufs=1, name=f"rden{u}")
            nc.vector.reciprocal_approx_fast(rden[:, :], dn[:, :])
            av = ps.tile([128, N_I], f32, tag="sc", bufs=3)
            for j in range(S_I):
                nc.tensor.matmul(
                    av[:, j * L:(j + 1) * L],
                    vbd[:, 2048 * g + 128 * j:2048 * g + 128 * (j + 1)],
                    ebuf[:, L * j:L * (j + 1)],
                    start=True, stop=True)
            nc.vector.tensor_mul(avn[g][:, :], av[:, :], rden[:, :])
        st["avn"] = avn

    def ph_proj():
        avn, xpf = st["avn"], st["xpf"]
        cl = slice(u * N_I, (u + 1) * N_I)
        for t in range(2):
            pp = ps.tile([128, N_I], f32, tag="mm", bufs=4)
            nc.tensor.matmul(pp[:, :], W[f"proj_{nm}"][:, 128 * t:128 * (t + 1)],
                             avn[0][:, :], start=True, stop=False)
            nc.tensor.matmul(pp[:, :], W[f"proj_{nm}"][:, 256 + 128 * t:256 + 128 * (t + 1)],
                             avn[1][:, :], start=False, stop=True)
            o2a = sb.tile([128, N_I], f32, tag=f"o2a{t}{u}", name=f"o2a{t}{u}")
            nc.vector.tensor_add(o2a[:, :], pp[:, :], xpf[t][:, :])
            nc.gpsimd.tensor_add(o2[t][:, cl], o2a[:, :],
                                 W[f"fb_{nm}"][:, N_I * t:N_I * (t + 1)])

    return [ph_embed, ph_stats, ph_qkv, ph_trans, ph_bd, ph_attn, ph_proj]


def _build_kernel(nc, tc, b_loc, loop_tok=None, static_dma=False):
    import contextlib
    ctx = contextlib.ExitStack()
    n_tok = b_loc * L
    if loop_tok is None:
        loop_tok = n_tok

    dram = {}
    for nm in ("x", "y"):
        dram[f"ic_{nm}"] = nc.dram_tensor(f"ic_{nm}", [52, n_tok], bf16, kind="ExternalInput").ap()
        dram[f"wemb_{nm}"] = nc.dram_tensor(f"wemb_{nm}", [52, DIM], bf16, kind="ExternalInput").ap()
        dram[f"wqkv_{nm}"] = nc.dram_tensor(f"wqkv_{nm}", [52, 768], bf16, kind="ExternalInput").ap()
        dram[f"wproj_{nm}"] = nc.dram_tensor(f"wproj_{nm}", [128, 512], bf16, kind="ExternalInput").ap()
        dram[f"fb_{nm}"] = nc.dram_tensor(f"fb_{nm}", [128, 2 * N_I], f32, kind="ExternalInput").ap()
    dram["ones_ln"] = nc.dram_tensor("ones_ln", [128, 128], bf16, kind="ExternalInput").ap()
    dram["ones_den"] = nc.dram_tensor("ones_den", [128, 128], bf16, kind="ExternalInput").ap()
    out_d = nc.dram_tensor("out", [2 * DIM, n_tok], bf16, kind="ExternalOutput").ap()

    const = ctx.enter_context(tc.tile_pool(name="const", bufs=1))
    sb = ctx.enter_context(tc.tile_pool(name="sb", bufs=1))
    ps = ctx.enter_context(tc.tile_pool(name="ps", bufs=2, space="PSUM"))

    W = {}
    for nm in ("x", "y"):
        for key, shp, dt in (("emb", [52, DIM], bf16), ("qkv", [52, 768], bf16),
                             ("proj", [128, 512], bf16), ("fb", [128, 2 * N_I], f32)):
            W[f"{key}_{nm}"] = const.tile(shp, dt, tag=f"{key}{nm}", name=f"{key}{nm}")
            nc.sync.dma_start(W[f"{key}_{nm}"][:, :], dram[f"w{key}_{nm}" if key != "fb" else f"fb_{nm}"])
    W["ones_ln"] = const.tile([128, 128], bf16, tag="ones_ln", name="ones_ln")
    nc.sync.dma_start(W["ones_ln"][:, :], dram["ones_ln"])
    W["ones_den"] = const.tile([128, 128], bf16, tag="ones_den", name="ones_den")
    nc.sync.dma_start(W["ones_den"][:, :], dram["ones_den"])
    W["eps256"] = const.tile([128, 1], f32, tag="eps256", name="eps256")
    nc.vector.memset(W["eps256"][:, :], 256.0 * LN_EPS)
    W["magic"] = const.tile([128, N_I], mybir.dt.uint32, tag="magic", name="magic")
    nc.vector.memset(W["magic"][:, :], 0x5f3759df)
    W["epsrow"] = const.tile([1, 128], bf16, tag="epsrow", name="epsrow")
    nc.vector.memset(W["epsrow"][:, :], LN_EPS)
    W["ones1"] = const.tile([1, N_I], bf16, tag="ones1", name="ones1")
    nc.vector.memset(W["ones1"][:, :], 1.0)

    # block-diag staging tiles: preamble-zeroed once; loop bodies overwrite
    # only the in-block 25/32-col regions, padding stays zero.
    kbd, vbd = {}, {}
    for u in range(U):
        for g in range(2):
            kbd[(g, u)] = const.tile([128, 128 * S_I], bf16, tag=f"kbd{g}{u}", name=f"kbd{g}{u}")
            nc.vector.memset(kbd[(g, u)][:, :], 0.0)
        vbd[u] = const.tile([128, 2 * 128 * S_I], bf16, tag=f"vbd{u}", name=f"vbd{u}")
        nc.vector.memset(vbd[u][:, :], 0.0)

    for nm in ("x", "y"):
        ob = 0 if nm == "x" else DIM
        with tc.For_i(0, loop_tok, TOK_B, name=f"chunks_{nm}", staggered_reset=True,
                      hint_engines=(mybir.EngineType.PE,)) as tok0:
            ic = sb.tile([52, TOK_B], bf16, tag="ic", bufs=2)
            if static_dma:
                nc.sync.dma_start(ic[:, :], dram[f"ic_{nm}"][:, 0:TOK_B])
            else:
                nc.sync.dma_start(ic[:, :], dram[f"ic_{nm}"][:, ds(tok0, TOK_B)])
            o2 = [sb.tile([128, TOK_B], bf16, tag=f"o2{t}", bufs=2, name=f"o2{t}") for t in range(2)]
            phases = [_slice_phases(nc, sb, ps, W, nm, ic, u,
                                    [kbd[(0, u)], kbd[(1, u)]], vbd[u], o2)
                      for u in range(U)]
            n_ph = len(phases[0])
            for slot in range(n_ph + STAGGER * (U - 1)):
                for u in range(U):
                    p = slot - STAGGER * u
                    if 0 <= p < n_ph:
                        phases[u][p]()
            for t in range(2):
                if static_dma:
                    nc.sync.dma_start(out_d[ob + 128 * t: ob + 128 * (t + 1), 0:TOK_B],
                                      o2[t][:, :])
                else:
                    nc.sync.dma_start(out_d[ob + 128 * t: ob + 128 * (t + 1), ds(tok0, TOK_B)],
                                      o2[t][:, :])
    ctx.close()


def _get_nc(b_loc, loop_tok=None, static_dma=False):
    key = (b_loc, loop_tok, static_dma)
    if key in _CACHE:
        return _CACHE[key]
    nc = bacc.Bacc("TRN2", target_bir_lowering=False, debug=False,
                   enable_asserts=False, num_devices=NCORES)
    with tile.TileContext(nc, trace_sim=False) as tc:
        _build_kernel(nc, tc, b_loc, loop_tok, static_dma)
    nc.compile()
    bass.Bass.finalize(nc)
    _CACHE[key] = nc
    return nc


def _in_maps(prep, b_loc, ncores):
    maps = []
    for c in range(ncores):
        s0 = c * b_loc
        m = {}
        for nm in ("x", "y"):
            ic = prep[f"ic_{nm}"][s0:s0 + b_loc].reshape(b_loc * L, 52).T
            m[f"ic_{nm}"] = _to_bf16(np.ascontiguousarray(ic))
            m[f"wemb_{nm}"] = _to_bf16(prep[f"wemb_{nm}"])
            m[f"wqkv_{nm}"] = _to_bf16(prep[f"wqkv_{nm}"])
            m[f"wproj_{nm}"] = _to_bf16(prep[f"wproj_{nm}"])
            m[f"fb_{nm}"] = prep[f"fb_{nm}"].astype(np.float32)
        m["ones_ln"] = _to_bf16(prep["ones_ln"])
        m["ones_den"] = _to_bf16(prep["ones_den"])
        maps.append(m)
    return maps


def kernel(**inputs):
    prep = _host_prep(inputs)
    nc = _get_nc(B_LOC)
    res = bass_utils.run_bass_kernel_spmd(nc, _in_maps(prep, B_LOC, NCORES),
                                          core_ids=list(range(NCORES)))
    outs = [res.results[c]["out"] for c in range(NCORES)]
    full = np.concatenate(
        [np.asarray(o, np.float32).reshape(2 * DIM, B_LOC, L).transpose(1, 0, 2)
         for o in outs], axis=0)
    return np.ascontiguousarray(full.reshape(B, 2 * DIM, 5, 5))
